# revision 7
# baseline (speedup 1.0000x reference)
"""TRN2 Bass kernel for nn_SynthesisLayer (StyleGAN-style modulated 3D conv).

Math: for each sample b
  styles = w[b] @ affine_weight.T / sqrt(512) + affine_bias          [Cin]
  wmod   = weight * styles[None,:,None]                              [Co,Ci,27]
  dcoef  = rsqrt(sum_{ci,k} wmod^2 + 1e-8)                           [Co]
  y      = dcoef * conv3d(x[b], wmod, pad=1) + noise_const*ns + bias
  out    = clip(lrelu(y)*sqrt(2), -256, 256)

Device implementation (per core): fp8(e4m3) DoubleRow matmul conv.
  Styles / modulation / demodulation fold into the weights on the host:
  wn = wmod * dcoef, scaled by 64 onto the e4m3 grid, split into
  W8 + RW (hi + residual); x splits into X8 + RX8/16. Per 264-col psum
  chunk the conv is one accumulation group of 41 DoubleRow matmuls
  (0.5 cy/row, 256-deep contraction):
    27 taps  (W8[k], W8[k]/16) x (X8@q_k, RX8@q_k)   -> w_hi * x (exact x)
    14 pairs (RW[2j], RW[2j+1]) x (X8@q_2j, X8@q_2j+1) -> residual w * X8
  Post: DVE stt (noise add + 1/64*sqrt2 scale), ACT Prelu (+bias, lrelu
  gain folded), fp16 out. Empirical rel err vs fp32 reference ~1.5e-3.

Sharding: 8 cores = 4 samples x 2 D-halves. Zero-padded input slab
[128, 3, NSLAB] (33-wide rows, 33-row slices, one-slice D halo; planes
X8 / RX8 / X8-copy -- the copy gives RW tap-pairs a non-overlapping
pair stride, which the PE requires). Output [128, 16*32*32] fp16,
host reassembles. No collectives.
"""

import math
import os
import sys

for _p in ("/opt/trn_rl_repo", "/root/.axon_site/_ro/trn_rl_repo"):
    if os.path.isdir(_p) and _p not in sys.path:
        sys.path.insert(0, _p)

import numpy as np
import ml_dtypes

import concourse.mybir as mybir
from concourse import bacc
from concourse.tile import TileContext
from concourse.bass_utils import run_bass_kernel_spmd
from concourse.ap import AP

P = 128          # Cin = Cout = 128
TAPS = 27        # 3x3x3
RES = 32
B = 4
ROW = 33         # padded row width  (32 real + 1 zero)
SLICE = ROW * ROW  # 1089 padded slice (32 real rows + 1 zero row)
LEAD = 34        # leading zero guard (one row + one elem)
NSLICES = 18     # 16 output slices + 1 halo each side
BODY = NSLICES * SLICE
NSLAB = LEAD + BODY + 46   # per-plane slab; max AP end = 19637
DHALF = 16                 # output D slices per core
NOUT = DHALF * RES * RES   # 16384
NCH = 264                  # psum chunk: 8 rows x 33 (pad col discarded)
ROWS_PER_CH = 8
WSCALE = 64.0              # weight grid scale (e4m3 sweet spot)
LRELU_ALPHA = 0.2
LRELU_GAIN = math.sqrt(2.0)

F8NP = ml_dtypes.float8_e4m3
BF16NP = ml_dtypes.bfloat16

f8 = mybir.dt.float8e4
f16 = mybir.dt.float16
bf16 = mybir.dt.bfloat16
f32 = mybir.dt.float32
AF = mybir.ActivationFunctionType
DR = mybir.MatmulPerfMode.DoubleRow

# tap k -> offset of its rhs window relative to the chunk base n0
def _tap_off(k):
    kd, r = divmod(k, 9)
    kh, kw = divmod(r, 3)
    return (kd - 1) * SLICE + (kh - 1) * ROW + (kw - 1)

TAP_OFF = [_tap_off(k) for k in range(TAPS)]

_NC_CACHE = None
LAST_EXEC_NS = None


def build_nc():
    nc = bacc.Bacc("TRN2", target_bir_lowering=False, debug=False, num_devices=8)

    xs = nc.dram_tensor("xs", [P, 3, NSLAB], f8, kind="ExternalInput")
    wt1 = nc.dram_tensor("wt1", [P, TAPS, 2, P], f8, kind="ExternalInput")
    wt2 = nc.dram_tensor("wt2", [P, 14, 2, P], f8, kind="ExternalInput")
    nz = nc.dram_tensor("nz", [1, NOUT], bf16, kind="ExternalInput")
    bcol = nc.dram_tensor("bcol", [P, 1], f32, kind="ExternalInput")
    acol = nc.dram_tensor("acol", [P, 1], f32, kind="ExternalInput")
    y = nc.dram_tensor("y", [P, NOUT], f16, kind="ExternalOutput")

    with TileContext(nc) as tc:
        with (
            tc.tile_pool(name="big", bufs=1) as big,
            tc.tile_pool(name="small", bufs=1) as small,
            tc.tile_pool(name="tmid", bufs=4) as tmid,
            tc.tile_pool(name="outp", bufs=4) as outp,
            tc.tile_pool(name="cpsum", bufs=8, space="PSUM") as cpsum,
        ):
            # ---- loads, ordered by first use on the serial DMA device ----
            bcol_sb = small.tile([P, 1], f32)
            nc.sync.dma_start(bcol_sb[:], bcol[:])
            acol_sb = small.tile([P, 1], f32)
            nc.sync.dma_start(acol_sb[:], acol[:])

            w1_sb = big.tile([P, TAPS, 2, P], f8)
            w2_sb = big.tile([P, 14, 2, P], f8)
            xt = big.tile([P, 3, NSLAB], f8)
            nz_sb = big.tile([P, NOUT], bf16)

            def seg(pl, s0, s1):
                a = s0 * SLICE
                bnd = min(NSLAB, LEAD + s1 * SLICE + 46)
                nc.sync.dma_start(xt[:, pl, a:bnd], xs[:, pl, a:bnd])

            # chunk 0 consumes taps in order: stage weights and the first
            # three slices so the PE starts ~3us in and never starves
            nc.sync.dma_start(w1_sb[:, :9], wt1[:, :9])
            seg(0, 0, 1)
            seg(1, 0, 1)
            nc.sync.dma_start(w1_sb[:, 9:18], wt1[:, 9:18])
            seg(0, 1, 2)
            seg(1, 1, 2)
            nc.sync.dma_start(w1_sb[:, 18:], wt1[:, 18:])
            seg(0, 2, 3)
            seg(1, 2, 3)
            nc.sync.dma_start(w2_sb[:], wt2[:])
            seg(2, 0, 3)
            # noise row broadcast (ns*sqrt2-scaled); first segment feeds the
            # early chunks' post ops, the rest trail the conv
            nc.sync.dma_start(nz_sb[:, :4096], nz[:, :4096].partition_broadcast(P))
            for s in range(3, NSLICES, 2):
                s1 = min(s + 2, NSLICES)
                for pl in range(3):
                    seg(pl, s, s1)
                if s in (5, 9, 13):
                    a = 4096 * ((s - 1) // 4)
                    nc.sync.dma_start(
                        nz_sb[:, a : a + 4096],
                        nz[:, a : a + 4096].partition_broadcast(P),
                    )

            # plane order is (RX8, X8, X8copy): every matmul pair step then
            # fits the PE's signed-16-bit AP step field (~NSLAB, not 2*NSLAB)

            def rw_rhs(n0, j):
                # pair (2j, 2j+1) read from planes (1, 2); j == 13 is the odd
                # tap 26 paired with a zero lhsT row
                k1 = 2 * j if j < 13 else 26
                k2 = 2 * j + 1 if j < 13 else 26
                q1 = n0 + TAP_OFF[k1]
                d = NSLAB + (TAP_OFF[k2] - TAP_OFF[k1])
                base = xt[:, 1, q1 : q1 + NCH]
                return AP(
                    base.tensor, base.offset,
                    [list(base.ap[0]), [d, 2], [1, NCH]],
                )

            # ---- main loop: 16 slices x 4 row-chunks ----
            for dl in range(1, DHALF + 1):
                for r0 in range(0, RES, ROWS_PER_CH):
                    n0 = LEAD + dl * SLICE + r0 * ROW
                    pt = cpsum.tile([P, NCH], f32, tag="conv")
                    for k in range(TAPS):
                        q = n0 + TAP_OFF[k]
                        nc.tensor.matmul(
                            pt[:], w1_sb[:, k], xt[:, 0:2, q : q + NCH],
                            start=(k == 0), stop=False, perf_mode=DR,
                        )
                    for j in range(14):
                        nc.tensor.matmul(
                            pt[:], w2_sb[:, j], rw_rhs(n0, j),
                            start=False, stop=(j == 13), perf_mode=DR,
                        )

                    off = (dl - 1) * RES * RES + r0 * RES
                    # t = psum * sqrt2/64 + noise  (pad cols dropped by views)
                    t = tmid.tile([P, ROWS_PER_CH, RES], f32, tag="t")
                    pview = pt[:].rearrange("p (r c) -> p r c", c=ROW)[:, :, :RES]
                    nzv = nz_sb[:, off : off + 256].rearrange(
                        "p (r c) -> p r c", c=RES
                    )
                    nc.vector.scalar_tensor_tensor(
                        t[:], pview, LRELU_GAIN / WSCALE, nzv,
                        mybir.AluOpType.mult, mybir.AluOpType.add,
                    )
                    # u = lrelu(t + bias*sqrt2), fp16 out
                    ot = outp.tile([P, 256], f16, tag="out")
                    nc.scalar.activation(
                        ot[:], t[:].rearrange("p r c -> p (r c)"), AF.Prelu,
                        bias=bcol_sb[:], scale=1.0, alpha=acol_sb[:],
                    )
                    nc.sync.dma_start(y[:, off : off + 256], ot[:])

    nc.compile()
    return nc


def _get_nc():
    global _NC_CACHE
    if _NC_CACHE is None:
        _NC_CACHE = build_nc()
    return _NC_CACHE


def _quant_weights(wn_b):
    """wn_b [Co,Ci,3,3,3] normalized weights -> (wt1, wt2) fp8 host arrays."""
    wt = np.ascontiguousarray(
        (wn_b * WSCALE).reshape(P, P, TAPS).transpose(1, 2, 0)
    ).astype(np.float32)  # [ci, k, co]
    W8 = wt.astype(F8NP)
    W8f = W8.astype(np.float32)
    RW = (wt - W8f).astype(F8NP)
    W8d16 = (W8f / 16.0).astype(F8NP)

    # rhs pair for the main taps is (RX8@plane0, X8@plane1), so the lhsT
    # pair order is (W8/16, W8)
    wt1 = np.zeros((P, TAPS, 2, P), F8NP)
    wt1[:, :, 0, :] = W8d16
    wt1[:, :, 1, :] = W8

    wt2 = np.zeros((P, 14, 2, P), F8NP)
    for j in range(13):
        wt2[:, j, 0, :] = RW[:, 2 * j]
        wt2[:, j, 1, :] = RW[:, 2 * j + 1]
    wt2[:, 13, 0, :] = RW[:, 26]
    return wt1, wt2


def _make_core_inputs(x, w, affine_weight, affine_bias, weight, noise_const,
                      noise_strength, bias):
    """Build the 8 per-core input maps (host-side sharding / quantization)."""
    w_dim = affine_weight.shape[1]
    styles = (w @ affine_weight.T) * (1.0 / math.sqrt(w_dim)) + affine_bias
    wmod = weight[None] * styles[:, None, :, None, None, None]
    dco = 1.0 / np.sqrt((wmod ** 2).sum(axis=(2, 3, 4, 5)) + 1e-8)
    wn = wmod * dco[:, :, None, None, None, None]          # [B,Co,Ci,3,3,3]

    X8 = x.astype(F8NP)
    RX8 = ((x - X8.astype(np.float32)) * 16.0).astype(F8NP)

    ns = float(noise_strength.reshape(-1)[0])
    bcol_host = (bias.reshape(P, 1) * LRELU_GAIN).astype(np.float32)
    acol_host = np.full((P, 1), LRELU_ALPHA, np.float32)

    wt_cache = [_quant_weights(wn[b]) for b in range(B)]

    in_maps = []
    for c in range(8):
        b, half = divmod(c, 2)
        d0 = DHALF * half
        slab = np.zeros((P, 3, NSLAB), F8NP)
        view = slab[:, :, LEAD : LEAD + BODY].reshape(P, 3, NSLICES, ROW, ROW)
        lo = max(0, d0 - 1)
        hi = min(RES, d0 + DHALF + 1)
        # padded slice s holds global slice d0-1+s
        view[:, 0, lo - (d0 - 1) : hi - (d0 - 1), :RES, :RES] = RX8[b, :, lo:hi]
        view[:, 1, lo - (d0 - 1) : hi - (d0 - 1), :RES, :RES] = X8[b, :, lo:hi]
        view[:, 2] = view[:, 1]
        nz_host = (
            noise_const[d0 : d0 + DHALF].reshape(1, NOUT) * (ns * LRELU_GAIN)
        ).astype(BF16NP)
        wt1_host, wt2_host = wt_cache[b]
        in_maps.append({
            "xs": slab,
            "wt1": wt1_host,
            "wt2": wt2_host,
            "nz": nz_host,
            "bcol": bcol_host,
            "acol": acol_host,
        })
    return in_maps


def kernel(x, w, affine_weight, affine_bias, weight, noise_const,
           noise_strength, bias):
    global LAST_EXEC_NS
    x = np.asarray(x, np.float32)
    w = np.asarray(w, np.float32)
    affine_weight = np.asarray(affine_weight, np.float32)
    affine_bias = np.asarray(affine_bias, np.float32)
    weight = np.asarray(weight, np.float32)
    noise_const = np.asarray(noise_const, np.float32)
    noise_strength = np.asarray(noise_strength, np.float32)
    bias = np.asarray(bias, np.float32)

    nc = _get_nc()
    in_maps = _make_core_inputs(
        x, w, affine_weight, affine_bias, weight, noise_const,
        noise_strength, bias,
    )
    trace = bool(os.environ.get("KERNEL_TRACE"))
    if trace:
        from concourse.bass_utils import axon_active

        if axon_active():
            try:  # axon NTFF capture needs the profile hook; absent in some pods
                from antenv.axon_hooks import get_axon_ntff_profile_hook  # noqa: F401
            except ImportError:
                trace = False
    res = run_bass_kernel_spmd(nc, in_maps, core_ids=list(range(8)), trace=trace)
    LAST_EXEC_NS = res.exec_time_ns

    out = np.empty((B, P, RES, RES, RES), np.float32)
    for c in range(8):
        b, half = divmod(c, 2)
        d0 = DHALF * half
        out[b, :, d0 : d0 + DHALF] = (
            res.results[c]["y"].astype(np.float32).reshape(P, DHALF, RES, RES)
        )
    return out


# revision 13
# speedup vs baseline: 1.2292x; 1.2292x over previous
"""TRN2 Bass kernel for nn_SynthesisLayer (StyleGAN-style modulated 3D conv).

Math: for each sample b
  styles = w[b] @ affine_weight.T / sqrt(512) + affine_bias          [Cin]
  wmod   = weight * styles[None,:,None]                              [Co,Ci,27]
  dcoef  = rsqrt(sum_{ci,k} wmod^2 + 1e-8)                           [Co]
  y      = dcoef * conv3d(x[b], wmod, pad=1) + noise_const*ns + bias
  out    = clip(lrelu(y)*sqrt(2), -256, 256)

Device implementation (per core): fp8(e4m3) DoubleRow matmul conv.
  Styles / modulation / demodulation fold into the weights on the host:
  wn = wmod * dcoef, scaled by 64 onto the e4m3 grid, split into
  W8 + RW (hi + residual); x splits into X8 + RX8/16. Per 264-col psum
  chunk the conv is one accumulation group of 41 DoubleRow matmuls
  (0.5 cy/row, 256-deep contraction):
    27 taps  (W8[k], W8[k]/16) x (X8@q_k, RX8@q_k)   -> w_hi * x (exact x)
    14 pairs (RW[2j], RW[2j+1]) x (X8@q_2j, X8@q_2j+1) -> residual w * X8
  Post: DVE stt (noise add + 1/64*sqrt2 scale), ACT Prelu (+bias, lrelu
  gain folded), fp16 out. Empirical rel err vs fp32 reference ~1.5e-3.

Sharding: 8 cores = 4 samples x 2 D-halves. Zero-padded input slab
[128, 3, NSLAB] (33-wide rows, 33-row slices, one-slice D halo; planes
X8 / RX8 / X8-copy -- the copy gives RW tap-pairs a non-overlapping
pair stride, which the PE requires). Output [128, 16*32*32] fp16,
host reassembles. No collectives.
"""

import math
import os
import sys

for _p in ("/opt/trn_rl_repo", "/root/.axon_site/_ro/trn_rl_repo"):
    if os.path.isdir(_p) and _p not in sys.path:
        sys.path.insert(0, _p)

import numpy as np
import ml_dtypes

import concourse.mybir as mybir
from concourse import bacc
from concourse.tile import TileContext
from concourse.bass_utils import run_bass_kernel_spmd
from concourse.ap import AP

P = 128          # Cin = Cout = 128
TAPS = 27        # 3x3x3
RES = 32
B = 4
ROW = 33         # padded row width  (32 real + 1 zero)
SLICE = ROW * ROW  # 1089 padded slice (32 real rows + 1 zero row)
LEAD = 34        # leading zero guard (one row + one elem)
NSLICES = 18     # 16 output slices + 1 halo each side
BODY = NSLICES * SLICE
NSLAB = LEAD + BODY + 46   # per-plane slab; max AP end = 19637
DHALF = 16                 # output D slices per core
NOUT = DHALF * RES * RES   # 16384
NCH = 264                  # psum chunk: 8 rows x 33 (pad col discarded)
ROWS_PER_CH = 8
WSCALE = 64.0              # weight grid scale (e4m3 sweet spot)
XCOMP = 17                 # taps 0..XCOMP-1 carry the fp8 x-residual term;
                           # the rest run tap-paired (rel err ~1.6e-2 < 2e-2)
MPAIRS = [(17, 18), (19, 20), (21, 22), (23, 24), (25, 26)]
LRELU_ALPHA = 0.2
LRELU_GAIN = math.sqrt(2.0)

F8NP = ml_dtypes.float8_e4m3
BF16NP = ml_dtypes.bfloat16

f8 = mybir.dt.float8e4
f16 = mybir.dt.float16
bf16 = mybir.dt.bfloat16
f32 = mybir.dt.float32
AF = mybir.ActivationFunctionType
DR = mybir.MatmulPerfMode.DoubleRow

# tap k -> offset of its rhs window relative to the chunk base n0
def _tap_off(k):
    kd, r = divmod(k, 9)
    kh, kw = divmod(r, 3)
    return (kd - 1) * SLICE + (kh - 1) * ROW + (kw - 1)

TAP_OFF = [_tap_off(k) for k in range(TAPS)]

_NC_CACHE = None
LAST_EXEC_NS = None


def build_nc():
    nc = bacc.Bacc("TRN2", target_bir_lowering=False, debug=False, num_devices=8)

    xs = nc.dram_tensor("xs", [P, 3, NSLAB], f8, kind="ExternalInput")
    wt1 = nc.dram_tensor("wt1", [P, XCOMP, 2, P], f8, kind="ExternalInput")
    wt2 = nc.dram_tensor("wt2", [P, 14, 2, P], f8, kind="ExternalInput")
    wt3 = nc.dram_tensor("wt3", [P, len(MPAIRS), 2, P], f8, kind="ExternalInput")
    nz = nc.dram_tensor("nz", [1, NOUT], f16, kind="ExternalInput")
    bcol = nc.dram_tensor("bcol", [P, 1], f32, kind="ExternalInput")
    acol = nc.dram_tensor("acol", [P, 1], f32, kind="ExternalInput")
    y = nc.dram_tensor("y", [P, NOUT], f16, kind="ExternalOutput")

    with TileContext(nc) as tc:
        with (
            tc.tile_pool(name="big", bufs=1) as big,
            tc.tile_pool(name="small", bufs=1) as small,
            tc.tile_pool(name="tmid", bufs=4) as tmid,
            tc.tile_pool(name="outp", bufs=4) as outp,
            tc.tile_pool(name="cpsum", bufs=8, space="PSUM") as cpsum,
        ):
            # ---- loads, ordered by first use on the serial DMA device ----
            bcol_sb = small.tile([P, 1], f32)
            nc.sync.dma_start(bcol_sb[:], bcol[:])
            acol_sb = small.tile([P, 1], f32)
            nc.sync.dma_start(acol_sb[:], acol[:])

            w1_sb = big.tile([P, XCOMP, 2, P], f8)
            w2_sb = big.tile([P, 14, 2, P], f8)
            w3_sb = big.tile([P, len(MPAIRS), 2, P], f8)
            xt = big.tile([P, 3, NSLAB], f8)
            nz_sb = big.tile([P, NOUT], f16)

            def seg(pl, s0, s1):
                a = s0 * SLICE
                bnd = min(NSLAB, LEAD + s1 * SLICE + 46)
                nc.sync.dma_start(xt[:, pl, a:bnd], xs[:, pl, a:bnd])

            # The PE consumes in order: chunk-0 native mains (plane0/1
            # prefixes + wt1), then the manual-AP pairs, whose dependency
            # span covers ALL of plane1 plus a plane2 prefix.  So: stage the
            # main prefixes, then finish plane1 in one big DMA, then weights
            # and the plane2 prefix; plane0/plane2 tails stream afterwards.
            nc.sync.dma_start(w1_sb[:, :9], wt1[:, :9])
            seg(0, 0, 1)
            seg(1, 0, 1)
            seg(0, 1, 2)
            seg(1, 1, 2)
            nc.sync.dma_start(w1_sb[:, 9:], wt1[:, 9:])
            seg(0, 2, 3)
            seg(1, 2, 3)
            nc.sync.dma_start(
                xt[:, 1, 3 * SLICE :], xs[:, 1, 3 * SLICE :]
            )  # finish plane1
            nc.sync.dma_start(w3_sb[:], wt3[:])
            nc.sync.dma_start(w2_sb[:], wt2[:])
            seg(2, 0, 3)
            # noise row broadcast (ns*sqrt2-scaled); first segment feeds the
            # early chunks' post ops, the rest trail the conv
            nc.sync.dma_start(nz_sb[:, :4096], nz[:, :4096].partition_broadcast(P))
            for s in range(3, NSLICES, 2):
                s1 = min(s + 2, NSLICES)
                seg(0, s, s1)
                seg(2, s, s1)
                if s in (5, 9, 13):
                    a = 4096 * ((s - 1) // 4)
                    nc.sync.dma_start(
                        nz_sb[:, a : a + 4096],
                        nz[:, a : a + 4096].partition_broadcast(P),
                    )

            # plane order is (RX8, X8, X8copy): every matmul pair step then
            # fits the PE's signed-16-bit AP step field (~NSLAB, not 2*NSLAB)

            def x8_pair_rhs(n0, k1, k2):
                # (X8@q_k1, X8copy@q_k2) from planes (1, 2): pair step
                # NSLAB + delta fits the PE's signed-16-bit AP step field
                q1 = n0 + TAP_OFF[k1]
                d = NSLAB + (TAP_OFF[k2] - TAP_OFF[k1])
                base = xt[:, 1, q1 : q1 + NCH]
                return AP(
                    base.tensor, base.offset,
                    [list(base.ap[0]), [d, 2], [1, NCH]],
                )

            RW_PAIRS = [(2 * j, 2 * j + 1) for j in range(13)] + [(26, 26)]

            # ---- main loop: 16 slices x 4 row-chunks ----
            for dl in range(1, DHALF + 1):
                for r0 in range(0, RES, ROWS_PER_CH):
                    n0 = LEAD + dl * SLICE + r0 * ROW
                    pt = cpsum.tile([P, NCH], f32, tag="conv")
                    for k in range(XCOMP):
                        q = n0 + TAP_OFF[k]
                        nc.tensor.matmul(
                            pt[:], w1_sb[:, k], xt[:, 0:2, q : q + NCH],
                            start=(k == 0), stop=False, perf_mode=DR,
                        )
                    for i, (k1, k2) in enumerate(MPAIRS):
                        nc.tensor.matmul(
                            pt[:], w3_sb[:, i], x8_pair_rhs(n0, k1, k2),
                            start=False, stop=False, perf_mode=DR,
                        )
                    for j, (k1, k2) in enumerate(RW_PAIRS):
                        nc.tensor.matmul(
                            pt[:], w2_sb[:, j], x8_pair_rhs(n0, k1, k2),
                            start=False, stop=(j == 13), perf_mode=DR,
                        )

                    off = (dl - 1) * RES * RES + r0 * RES
                    # t = psum * sqrt2/64 + noise  (pad cols dropped by views)
                    t = tmid.tile([P, ROWS_PER_CH, RES], f32, tag="t")
                    pview = pt[:].rearrange("p (r c) -> p r c", c=ROW)[:, :, :RES]
                    nzv = nz_sb[:, off : off + 256].rearrange(
                        "p (r c) -> p r c", c=RES
                    )
                    nc.vector.scalar_tensor_tensor(
                        t[:], pview, LRELU_GAIN / WSCALE, nzv,
                        mybir.AluOpType.mult, mybir.AluOpType.add,
                    )
                    # u = lrelu(t + bias*sqrt2), fp16 out
                    ot = outp.tile([P, 256], f16, tag="out")
                    nc.scalar.activation(
                        ot[:], t[:].rearrange("p r c -> p (r c)"), AF.Prelu,
                        bias=bcol_sb[:], scale=1.0, alpha=acol_sb[:],
                    )
                    nc.sync.dma_start(y[:, off : off + 256], ot[:])

    nc.compile()
    return nc


def _get_nc():
    global _NC_CACHE
    if _NC_CACHE is None:
        _NC_CACHE = build_nc()
    return _NC_CACHE


def _quant_weights(wn_b):
    """wn_b [Co,Ci,3,3,3] normalized weights -> (wt1, wt2, wt3) fp8 arrays."""
    wt = np.ascontiguousarray(
        (wn_b * WSCALE).reshape(P, P, TAPS).transpose(1, 2, 0)
    ).astype(np.float32)  # [ci, k, co]
    W8 = wt.astype(F8NP)
    W8f = W8.astype(np.float32)
    RW = (wt - W8f).astype(F8NP)
    W8d16 = (W8f / 16.0).astype(F8NP)

    # rhs pair for the x-comped taps is (RX8@plane0, X8@plane1), so the
    # lhsT pair order is (W8/16, W8)
    wt1 = np.zeros((P, XCOMP, 2, P), F8NP)
    wt1[:, :, 0, :] = W8d16[:, :XCOMP]
    wt1[:, :, 1, :] = W8[:, :XCOMP]

    wt2 = np.zeros((P, 14, 2, P), F8NP)
    for j in range(13):
        wt2[:, j, 0, :] = RW[:, 2 * j]
        wt2[:, j, 1, :] = RW[:, 2 * j + 1]
    wt2[:, 13, 0, :] = RW[:, 26]

    wt3 = np.zeros((P, len(MPAIRS), 2, P), F8NP)
    for i, (k1, k2) in enumerate(MPAIRS):
        wt3[:, i, 0, :] = W8[:, k1]
        wt3[:, i, 1, :] = W8[:, k2]
    return wt1, wt2, wt3


def _make_core_inputs(x, w, affine_weight, affine_bias, weight, noise_const,
                      noise_strength, bias):
    """Build the 8 per-core input maps (host-side sharding / quantization)."""
    w_dim = affine_weight.shape[1]
    styles = (w @ affine_weight.T) * (1.0 / math.sqrt(w_dim)) + affine_bias
    wmod = weight[None] * styles[:, None, :, None, None, None]
    dco = 1.0 / np.sqrt((wmod ** 2).sum(axis=(2, 3, 4, 5)) + 1e-8)
    wn = wmod * dco[:, :, None, None, None, None]          # [B,Co,Ci,3,3,3]

    X8 = x.astype(F8NP)
    RX8 = ((x - X8.astype(np.float32)) * 16.0).astype(F8NP)

    ns = float(noise_strength.reshape(-1)[0])
    bcol_host = (bias.reshape(P, 1) * LRELU_GAIN).astype(np.float32)
    acol_host = np.full((P, 1), LRELU_ALPHA, np.float32)

    wt_cache = [_quant_weights(wn[b]) for b in range(B)]

    in_maps = []
    for c in range(8):
        b, half = divmod(c, 2)
        d0 = DHALF * half
        slab = np.zeros((P, 3, NSLAB), F8NP)
        view = slab[:, :, LEAD : LEAD + BODY].reshape(P, 3, NSLICES, ROW, ROW)
        lo = max(0, d0 - 1)
        hi = min(RES, d0 + DHALF + 1)
        # padded slice s holds global slice d0-1+s
        view[:, 0, lo - (d0 - 1) : hi - (d0 - 1), :RES, :RES] = RX8[b, :, lo:hi]
        view[:, 1, lo - (d0 - 1) : hi - (d0 - 1), :RES, :RES] = X8[b, :, lo:hi]
        view[:, 2] = view[:, 1]
        nz_host = (
            noise_const[d0 : d0 + DHALF].reshape(1, NOUT) * (ns * LRELU_GAIN)
        ).astype(np.float16)
        wt1_host, wt2_host, wt3_host = wt_cache[b]
        in_maps.append({
            "xs": slab,
            "wt1": wt1_host,
            "wt2": wt2_host,
            "wt3": wt3_host,
            "nz": nz_host,
            "bcol": bcol_host,
            "acol": acol_host,
        })
    return in_maps


def kernel(x, w, affine_weight, affine_bias, weight, noise_const,
           noise_strength, bias):
    global LAST_EXEC_NS
    x = np.asarray(x, np.float32)
    w = np.asarray(w, np.float32)
    affine_weight = np.asarray(affine_weight, np.float32)
    affine_bias = np.asarray(affine_bias, np.float32)
    weight = np.asarray(weight, np.float32)
    noise_const = np.asarray(noise_const, np.float32)
    noise_strength = np.asarray(noise_strength, np.float32)
    bias = np.asarray(bias, np.float32)

    nc = _get_nc()
    in_maps = _make_core_inputs(
        x, w, affine_weight, affine_bias, weight, noise_const,
        noise_strength, bias,
    )
    trace = bool(os.environ.get("KERNEL_TRACE"))
    if trace:
        from concourse.bass_utils import axon_active

        if axon_active():
            try:  # axon NTFF capture needs the profile hook; absent in some pods
                from antenv.axon_hooks import get_axon_ntff_profile_hook  # noqa: F401
            except ImportError:
                trace = False
    res = run_bass_kernel_spmd(nc, in_maps, core_ids=list(range(8)), trace=trace)
    LAST_EXEC_NS = res.exec_time_ns

    out = np.empty((B, P, RES, RES, RES), np.float32)
    for c in range(8):
        b, half = divmod(c, 2)
        d0 = DHALF * half
        out[b, :, d0 : d0 + DHALF] = (
            res.results[c]["y"].astype(np.float32).reshape(P, DHALF, RES, RES)
        )
    return out
